# revision 35
# baseline (speedup 1.0000x reference)
"""Paged GQA decode attention (sparse_attention) on 8 TRN2 NeuronCores.

Sharding: batch (64 seqs) split across 8 cores, 8 seqs/core. Each core
receives a compacted paged-KV shard holding only the (deduplicated) blocks
referenced by its sequences, plus remapped gather/scatter index tensors.
All per-call data flows through input tensors, so one SPMD program serves
every core and every call.

Per-core device program:
  1. indirect-DMA scatter of the new k/v rows into the DRAM shard
     (the paged-cache update).
  2. per (seq, half): one dma_gather pulls 512 token-rows (4KB each) of
     K and of V from the shard into SBUF in token-major layout.
  3. ACT casts K f32->bf16; an SBUF-source transposing dma_gather then
     produces K^T [d=128 partitions, kv, tokens] for the PE.
  4. PE: scores^T chunks [128 tok, 32 heads] = (K^T_k chunk)^T @ q^T_k;
     ACT exp (no max-subtraction needed: scores ~ N(0,1), |s|<~6);
     softmax denominators via ones-matmul; PV via V-stationary matmuls
     accumulating o^T [128 d, 32 heads] in PSUM.
  5. reciprocal + PE transpose + scale -> out row [32, 128] f32.
"""

import sys

import numpy as np

for _p in ("/opt/trn_rl_repo",):
    if _p not in sys.path:
        sys.path.insert(0, _p)

# ---- problem constants (hardcoded from the spec) ----
NUM_HEADS = 32
HEAD_DIM = 128
NUM_KV = 8
GROUP = NUM_HEADS // NUM_KV  # 4
SCALE = 0.08838834764831845
NUM_BLOCKS = 4096
BLOCK_SIZE = 16
BLOCKS_PER_SEQ = 64
BATCH = 64
NCORES = 8
SEQ_PER_CORE = BATCH // NCORES  # 8
S = BLOCKS_PER_SEQ * BLOCK_SIZE  # 1024 tokens per seq
KV_FLAT = NUM_KV * HEAD_DIM  # 1024 floats per token-row
R = SEQ_PER_CORE * BLOCKS_PER_SEQ  # 512 shard blocks (padded max)
ROWS = R * BLOCK_SIZE  # 8192 shard token-rows
HALF = 512  # tokens processed per pipeline stage
NH = HALF // 128  # 4 chunks of 128 tokens per half
QUAD = 1  # tokens per gathered row; {1,2,4} supported (4KB rows fastest on HW)
TIDXC = HALF // 16  # 32 idx columns for the transpose gather

LAST_RESULTS = None  # BassKernelResults of the most recent run (for test.py)

_PROG = None


def _build_program(repeat=1, mode="full", quad=QUAD):
    QROWS = HALF // quad
    IDXC = QROWS // 16
    MID = QROWS // 128
    import concourse.bass as bass
    import concourse.bacc as bacc
    import concourse.mybir as mybir
    import concourse.tile as tile
    from concourse import library_config
    from concourse.masks import make_identity
    from concourse.tile_rust import add_dep_helper
    from contextlib import ExitStack

    f32 = mybir.dt.float32
    bf16 = mybir.dt.bfloat16
    i16 = mybir.dt.int16
    i32 = mybir.dt.int32

    nc = bacc.Bacc("TRN2", target_bir_lowering=False, debug=False)
    q_d = nc.declare_dram_parameter("q", [SEQ_PER_CORE, NUM_HEADS, HEAD_DIM], f32, isOutput=False)
    kn_d = nc.declare_dram_parameter("knew", [BATCH, KV_FLAT], f32, isOutput=False)
    vn_d = nc.declare_dram_parameter("vnew", [BATCH, KV_FLAT], f32, isOutput=False)
    ks_d = nc.declare_dram_parameter("kshard", [ROWS, KV_FLAT], f32, isOutput=False)
    vs_d = nc.declare_dram_parameter("vshard", [ROWS, KV_FLAT], f32, isOutput=False)
    gx_d = nc.declare_dram_parameter("gidx", [128, SEQ_PER_CORE * 2 * IDXC], i16, isOutput=False)
    tx_d = nc.declare_dram_parameter("tidx", [128, TIDXC], i16, isOutput=False)
    sx_d = nc.declare_dram_parameter("sidx", [128, 1], i32, isOutput=False)
    out_d = nc.declare_dram_parameter("out", [SEQ_PER_CORE, NUM_HEADS * HEAD_DIM], f32, isOutput=True)

    out_view = out_d[:].rearrange("s (h d) -> s h d", d=HEAD_DIM)

    with tile.TileContext(nc) as tc, ExitStack() as ctx:
        const = ctx.enter_context(tc.tile_pool(name="const", bufs=1))
        knat = ctx.enter_context(tc.tile_pool(name="knat", bufs=2))
        vnat = ctx.enter_context(tc.tile_pool(name="vnat", bufs=2))
        k16p = ctx.enter_context(tc.tile_pool(name="k16p", bufs=3))
        v16p = ctx.enter_context(tc.tile_pool(name="v16p", bufs=3))
        ktp = ctx.enter_context(tc.tile_pool(name="ktp", bufs=3))
        prp = ctx.enter_context(tc.tile_pool(name="prp", bufs=6))
        sbm = ctx.enter_context(tc.tile_pool(name="sbm", bufs=3))
        scp = ctx.enter_context(tc.tile_pool(name="scp", bufs=2, space="PSUM"))
        otp = ctx.enter_context(tc.tile_pool(name="otp", bufs=2, space="PSUM"))
        smp = ctx.enter_context(tc.tile_pool(name="smp", bufs=1, space="PSUM"))
        trp = ctx.enter_context(tc.tile_pool(name="trp", bufs=3, space="PSUM"))

        nc.gpsimd.load_library(library_config.mlp)

        ks_q = ks_d[:].rearrange("(r q) e -> r (q e)", q=quad)
        vs_q = vs_d[:].rearrange("(r q) e -> r (q e)", q=quad)

        identity = const.tile([128, 128], f32)
        make_identity(nc, identity[:])
        identity16 = const.tile([128, 128], bf16)
        nc.vector.tensor_copy(identity16[:], identity[:])
        ones = const.tile([128, 1], f32)
        nc.gpsimd.memset(ones[:], 1.0)
        ones16 = const.tile([128, 1], bf16)
        nc.gpsimd.memset(ones16[:], 1.0)
        tidx = const.tile([128, TIDXC], i16)
        nc.sync.dma_start(tidx[:], tx_d[:])
        sidx = const.tile([128, 1], i32)
        nc.sync.dma_start(sidx[:], sx_d[:])
        gidx = const.tile([128, SEQ_PER_CORE * 2 * IDXC], i16)
        nc.sync.dma_start(gidx[:], gx_d[:])

        # ---- paged-cache update: scatter new k/v rows into the DRAM shard ----
        knt = const.tile([128, KV_FLAT], f32)
        vnt = const.tile([128, KV_FLAT], f32)
        nc.gpsimd.memset(knt[:], 0.0)
        nc.gpsimd.memset(vnt[:], 0.0)
        nc.sync.dma_start(knt[:BATCH, :], kn_d[:])
        nc.sync.dma_start(vnt[:BATCH, :], vn_d[:])
        sc_k = nc.gpsimd.indirect_dma_start(
            out=ks_d[:],
            out_offset=bass.IndirectOffsetOnAxis(ap=sidx[:, :1], axis=0),
            in_=knt[:],
            in_offset=None,
            bounds_check=ROWS - 1,
            oob_is_err=False,
        )
        sc_v = nc.gpsimd.indirect_dma_start(
            out=vs_d[:],
            out_offset=bass.IndirectOffsetOnAxis(ap=sidx[:, :1], axis=0),
            in_=vnt[:],
            in_offset=None,
            bounds_check=ROWS - 1,
            oob_is_err=False,
        )

        loop_ctx = tc.For_i(0, repeat, 1) if repeat > 1 else None
        if loop_ctx is not None:
            loop_ctx.__enter__()
        for s in range(SEQ_PER_CORE):
            if mode != "gathers":
                # q^T prep: [32,128] -> PE transpose -> scale+cast -> [128d, 32h] bf16
                qs = sbm.tile([NUM_HEADS, HEAD_DIM], f32, tag="qs")
                nc.sync.dma_start(qs[:], q_d[s])
                qtp = trp.tile([HEAD_DIM, NUM_HEADS], f32, tag="tr")
                nc.tensor.transpose(qtp[:], qs[:], identity[:NUM_HEADS, :NUM_HEADS])
                qT = sbm.tile([HEAD_DIM, NUM_HEADS], bf16, tag="qT")
                nc.scalar.mul(qT[:], qtp[:], SCALE)

                sums = smp.tile([NUM_HEADS, 1], f32)
                oT = otp.tile([HEAD_DIM, NUM_HEADS], f32)

            for h in range(2):
                goff = (s * 2 + h) * IDXC
                kna = knat.tile([128, MID, quad * KV_FLAT], f32)
                g1 = nc.gpsimd.dma_gather(
                    out_ap=kna[:],
                    in_ap=ks_q,
                    idxs_ap=gidx[:, goff : goff + IDXC],
                    num_idxs=QROWS,
                    num_idxs_reg=QROWS,
                    elem_size=quad * KV_FLAT,
                )
                add_dep_helper(g1.ins, sc_k.ins, reason="cache update before K gather")
                vna = vnat.tile([128, MID, quad * KV_FLAT], f32)
                g2 = nc.gpsimd.dma_gather(
                    out_ap=vna[:],
                    in_ap=vs_q,
                    idxs_ap=gidx[:, goff : goff + IDXC],
                    num_idxs=QROWS,
                    num_idxs_reg=QROWS,
                    elem_size=quad * KV_FLAT,
                )
                add_dep_helper(g2.ins, sc_v.ins, reason="cache update before V gather")

                if mode == "gathers":
                    continue
                kt = ktp.tile([128, NUM_KV, HALF], bf16)
                if mode == "pet":
                    # PE-transpose path: cast K to bf16 on ACT, transpose each
                    # [128 tok, 128 d] slice on the PE, evacuate PSUM->SBUF on DVE.
                    k16 = k16p.tile([128, MID, quad * KV_FLAT], bf16)
                    nc.scalar.copy(k16[:], kna[:])
                    v16 = v16p.tile([128, MID, quad * KV_FLAT], bf16)
                    nc.vector.tensor_copy(v16[:], vna[:])
                    for c in range(NH):
                        for kv in range(NUM_KV):
                            ktr = trp.tile([HEAD_DIM, 128], bf16, tag="tr")
                            nc.tensor.transpose(
                                ktr[:],
                                k16[:, c // quad,
                                    (c % quad) * KV_FLAT + kv * HEAD_DIM
                                    : (c % quad) * KV_FLAT + (kv + 1) * HEAD_DIM],
                                identity16[:],
                            )
                            nc.vector.tensor_copy(kt[:, kv, c * 128 : (c + 1) * 128], ktr[:])
                elif mode != "noxpose":
                    k16 = k16p.tile([128, MID, quad * KV_FLAT], bf16)
                    nc.scalar.copy(k16[:], kna[:])
                    nc.gpsimd.dma_gather(
                        out_ap=kt[:],
                        in_ap=k16[:],
                        idxs_ap=tidx[:],
                        num_idxs=HALF,
                        num_idxs_reg=HALF,
                        elem_size=KV_FLAT,
                        transpose=True,
                        sbuf_tokens_per_rank=128,
                        sbuf_free_dim_per_rank=KV_FLAT * 2,
                    )

                for c in range(NH):
                    gc = h * NH + c
                    sc = scp.tile([128, NUM_HEADS], f32)
                    for kv in range(NUM_KV):
                        nc.tensor.matmul(
                            sc[:, kv * GROUP : (kv + 1) * GROUP],
                            lhsT=kt[:, kv, c * 128 : (c + 1) * 128],
                            rhs=qT[:, kv * GROUP : (kv + 1) * GROUP],
                            start=(kv == 0),
                            stop=(kv == NUM_KV - 1),
                            skip_group_check=True,
                        )
                    pr = prp.tile([128, NUM_HEADS], bf16 if mode == "pet" else f32)
                    nc.scalar.activation(pr[:], sc[:], mybir.ActivationFunctionType.Exp)
                    nc.tensor.matmul(
                        sums[:],
                        lhsT=pr[:],
                        rhs=ones16[:] if mode == "pet" else ones[:],
                        start=(gc == 0),
                        stop=(gc == 2 * NH - 1),
                        skip_group_check=True,
                    )
                    vsrc = v16 if mode == "pet" else vna
                    for kv in range(NUM_KV):
                        nc.tensor.matmul(
                            oT[:, kv * GROUP : (kv + 1) * GROUP],
                            lhsT=vsrc[:, c // quad, (c % quad) * KV_FLAT + kv * HEAD_DIM : (c % quad) * KV_FLAT + (kv + 1) * HEAD_DIM],
                            rhs=pr[:, kv * GROUP : (kv + 1) * GROUP],
                            start=(gc == 0 and kv == 0),
                            stop=(gc == 2 * NH - 1 and kv == NUM_KV - 1),
                            skip_group_check=True,
                        )

            if mode == "gathers":
                continue
            inv = sbm.tile([NUM_HEADS, 1], f32, tag="inv")
            nc.vector.reciprocal(inv[:], sums[:])
            oTs = sbm.tile([HEAD_DIM, NUM_HEADS], f32, tag="oTs")
            nc.scalar.copy(oTs[:], oT[:])
            op = trp.tile([NUM_HEADS, HEAD_DIM], f32, tag="tr")
            nc.tensor.transpose(op[:], oTs[:], identity[:])
            ob = sbm.tile([NUM_HEADS, HEAD_DIM], f32, tag="ob")
            nc.vector.tensor_scalar_mul(ob[:], op[:], inv[:, :1])
            nc.sync.dma_start(out_view[s], ob[:])

        if loop_ctx is not None:
            loop_ctx.__exit__(None, None, None)

    nc.compile()
    return nc


def _get_program():
    global _PROG
    if _PROG is None:
        _PROG = _build_program(mode="pet")
    return _PROG


def _wrap_idx(vec):
    """Arrange a length-(16*C) index vector as the [16, C] SWDGE tile layout
    (idx i at [i % 16, i // 16]) and replicate to 128 partitions."""
    c = len(vec) // 16
    t = np.asarray(vec, np.int16).reshape(c, 16).T  # [16, C]
    return np.tile(t, (8, 1))  # [128, C]


def build_in_maps(q, k, v, k_cache, v_cache, slot_mapping, block_tables, quad=QUAD):
    q = np.ascontiguousarray(np.asarray(q, np.float32))
    knew = np.ascontiguousarray(np.asarray(k, np.float32).reshape(BATCH, KV_FLAT))
    vnew = np.ascontiguousarray(np.asarray(v, np.float32).reshape(BATCH, KV_FLAT))
    kc = np.asarray(k_cache, np.float32).reshape(NUM_BLOCKS, BLOCK_SIZE * KV_FLAT)
    vc = np.asarray(v_cache, np.float32).reshape(NUM_BLOCKS, BLOCK_SIZE * KV_FLAT)
    slot_mapping = np.asarray(slot_mapping, np.int64)
    block_tables = np.asarray(block_tables, np.int64)

    tidx = _wrap_idx(np.arange(HALF))  # identity: transpose-gather slot order

    # main-gather quad rows: position i of (seq, half) -> tokens h*512 + quad*i ..
    i_arr = np.arange(HALF // quad)
    tblpos = i_arr // (BLOCK_SIZE // quad)  # block-table column within the half
    qwb = i_arr % (BLOCK_SIZE // quad)  # quad within block

    in_maps = []
    for core in range(NCORES):
        seqs = slice(core * SEQ_PER_CORE, (core + 1) * SEQ_PER_CORE)
        bt = block_tables[seqs]  # [8, 64]
        uniq = np.unique(bt)
        nu = len(uniq)
        assert nu <= R
        pos = np.full(NUM_BLOCKS, -1, np.int64)
        pos[uniq] = np.arange(nu)

        kshard = np.zeros((ROWS, KV_FLAT), np.float32)
        vshard = np.zeros((ROWS, KV_FLAT), np.float32)
        kshard[: nu * BLOCK_SIZE] = kc[uniq].reshape(-1, KV_FLAT)
        vshard[: nu * BLOCK_SIZE] = vc[uniq].reshape(-1, KV_FLAT)

        # main-gather rows at quad granularity: shard quad-row of position i
        # of (seq ls, half h) = pos[bt[ls, h*32 + i//4]]*4 + i%4
        gcols = []
        for ls in range(SEQ_PER_CORE):
            for h in range(2):
                blk = pos[bt[ls, h * (HALF // BLOCK_SIZE) + tblpos]]
                assert blk.min() >= 0
                gcols.append(_wrap_idx(blk * (BLOCK_SIZE // quad) + qwb))
        gidx = np.concatenate(gcols, axis=1).astype(np.int16)  # [128, 8*2*8]

        # scatter rows: new-token row i lands at flat cache row slot_mapping[i]
        sidx = np.full((128, 1), 1 << 20, np.int32)
        for i in range(BATCH):
            sl = int(slot_mapping[i])
            b, off = divmod(sl, BLOCK_SIZE)
            if pos[b] >= 0:
                sidx[i, 0] = pos[b] * BLOCK_SIZE + off

        in_maps.append(
            {
                "q": np.ascontiguousarray(q[seqs]),
                "knew": knew,
                "vnew": vnew,
                "kshard": kshard,
                "vshard": vshard,
                "gidx": np.ascontiguousarray(gidx),
                "tidx": np.ascontiguousarray(tidx),
                "sidx": sidx,
            }
        )
    return in_maps


def kernel(q, k, v, k_cache, v_cache, slot_mapping, block_tables):
    from concourse.bass_utils import run_bass_kernel_spmd

    global LAST_RESULTS
    in_maps = build_in_maps(q, k, v, k_cache, v_cache, slot_mapping, block_tables)
    nc = _get_program()
    LAST_RESULTS = run_bass_kernel_spmd(nc, in_maps, core_ids=list(range(NCORES)))
    out = np.concatenate([LAST_RESULTS.results[i]["out"] for i in range(NCORES)], axis=0)
    return np.ascontiguousarray(out.astype(np.float32))
